# revision 55
# baseline (speedup 1.0000x reference)
"""Trainium2 8-core attention kernel for nn_Attention_14104672600564.

Problem: x[4,128,64,64] f32; wq/wk/wv/wo [128,128]; bo[128].
  per (b,h): sim = (wq x)^T (wk x) * d^-.5 ; attn = softmax(sim) ; out_h = attn @ (wv x)^T
  out = wo @ concat_h(out_h^T) + bo

Sharding: 16 independent (batch, head) attention problems -> 8 cores; each core
gets one batch and one head-pair; the host unshards by summing the two cores of
each batch.

v4.2 design, 253.0us (v4/v4.1 with accum-DMA C-class: 294/286us — worse;
v3: 256.2us; v2: 250us bf16; v1: 284us):
  - Strip classes per 32 j-strips, alternating ACT/DVE by parity so neither
    exp engine sees bursts: B (16 even js, ACT exp -> bf16 pt -> bf16 AV),
    D (16 odd js, DVE custom poly expm1 -> fp8 -> DoubleRow AV). Both exp
    engines run ~100% in steady state; the equilibrium wall is their
    throughput, so nothing extra may ride them mid-stream.
  - The C-class experiment (gpsimd casting accum DMA adds ACT's bf16 exp
    onto a -1-prefilled fp8 tile -> fp8 expm1 -> DR AV) is HW-correct and
    numerically ideal, but its multi-queue chain (ACT -> Pool issue ~1us ->
    sw ring ~3.1us -> DR AV) jitters the saturated engines; PE gaps grew
    2x more than the ~10us of PE work it saved. Disabled via USE_CSET.
    Partition-offset matmul inputs (base>0) pass BIR verify but CRASH the
    runtime — packed Q/K projection disabled via USE_PACKED_PROJ.
  - corr (the expm1 "+1" restoration column) via a single ones-stationary
    accumulation: out[0,m] = sum_j vts[j,m] over all DR strips (one
    LoadStationary, 66-col matmuls), then a DRAM bounce scatters the
    [1,66] row into the per-partition corrS[33,2] bias layout.
  - Tail (saves ~6us): the last two groups broadcast their reciprocal via a
    contract-1 f32 PE matmul (ones[1,33] x rec[1,512]) into the retired av
    PSUM banks instead of the DRAM round-trip — the PE is idle in the drain
    and this skips two DMA+semaphore hops; their normalize muls run on DVE
    reading the PSUM broadcast; the final po fans its two output DMAs
    across the sync+scalar queues.
  - Prologue: wqkT first on sync, x chunks 0-1 on the scalar queue so the
    first projections and the rest of x land in parallel.
  - Flat pipeline, AVLAG queue, PO_DELAY borrowed sim slots as in v3.
"""

import sys

sys.path.insert(0, "/opt/trn_rl_repo")

import numpy as np
import ml_dtypes

import concourse.bass as bass
import concourse.bacc as bacc
import concourse.tile as tile
from concourse import mybir
import concourse.bass_utils as _bass_utils
from concourse.bass_utils import run_bass_kernel_spmd

# ---- custom DVE exp ops (degree-4 polynomial, |x| <= ~0.5) -----------------
import concourse.dve_ops as _dvo
from concourse.dve_ops import DveOp as _DveOp
from concourse.dve_spec import (
    Spec as _Spec,
    Src0 as _Src0,
    C0 as _C0,
    C1 as _C1,
    C2 as _C2,
    One as _One,
    lower as _dve_lower,
)
from concourse.dve_uop import DveOpSpec as _DveOpSpec

# fit of exp(x) on [-0.45, 0.45] with c0=c1=1 fixed:
# exp(x) ~ 1 + x*(1 + x*(c2 + x*(c3 + x*c4))); max rel err 3.5e-5
_EXPC2, _EXPC3, _EXPC4 = 0.50019703, 0.16796468, 0.04051121


def _register_op(name, body, reference):
    for op in _dvo.OPS:
        if op.name == name:
            return op
    spec = _Spec(body=body, reference=reference)
    row = _dvo._CUSTOM_DVE_ROW_BASE + len(_dvo.OPS)
    shas = {}
    for ver in ("v3", "v4"):
        uops = _dve_lower(spec, ver=ver)
        shas[ver] = _DveOpSpec(
            name=name, opcode=row, uops=uops, rd1_en=False
        ).sha(ver)
    op = _DveOp(name, spec, subdim=False, uops_sha=shas)
    _dvo.OPS.append(op)
    _dvo._SUB_OPCODE_FOR_NAME[name] = row
    return op


def _register_exp_ops():
    t = _Src0 * _C2 + _C1
    t = t * _Src0 + _C0
    t = t * _Src0 + _One
    exp_op = _register_op(
        "EXP_POLY4_ANT",
        t * _Src0 + _One,
        lambda in0, in1, s0, s1, imm2: (
            (((imm2 * in0 + s1) * in0 + s0) * in0 + 1.0) * in0 + 1.0
        ),
    )
    em1_op = _register_op(
        "EXPM1_POLY4_ANT",
        t * _Src0,
        lambda in0, in1, s0, s1, imm2: (
            (((imm2 * in0 + s1) * in0 + s0) * in0 + 1.0) * in0
        ),
    )
    return exp_op, em1_op


_EXP_OP, _EXPM1_OP = _register_exp_ops()


BF16 = mybir.dt.bfloat16
F32 = mybir.dt.float32
FP8 = mybir.dt.float8e4
DRMODE = mybir.MatmulPerfMode.DoubleRow
ADD = mybir.AluOpType.add

HEADS = 4
DH = 32  # dim per head
C = 128  # channels
N = 4096  # tokens (64*64)
IC = 512  # i-chunk
NICH = N // IC  # 8
JS = 128  # j-strip
NJS = N // JS  # 32
VBLK = 2 * (DH + 1)  # 66: [1|Vh0|1|Vh1] per j-strip

# feature toggles (fallbacks are the v3-proven paths)
# The accum-DMA C-class is HW-validated but its multi-queue conversion chain
# (ACT -> Pool issue -> sw ring -> DR AV) adds enough jitter to the ~100%%-
# loaded exp engines that PE gaps grow more than the ~10us of PE work it
# saves (294/286us vs 256us benches). Disabled.
USE_CSET = False
# partition-offset matmul inputs crash the runtime (probe2 B): keep the
# v3-style zero-padded per-head projection.
USE_PACKED_PROJ = False
USE_PE_BCAST = True  # tail groups broadcast recip via PE instead of DMA bounce
USE_CORR_ONES = True  # corr via ones-stationary matmul + bounce transpose

# strip classes: even js -> ACT exp, odd js -> DVE expm1 (alternation keeps
# both exp engines fed each slot). Within the ACT evens, B-class strips stay
# bf16 (their AV runs bf16, carrying start=True on js0); C-class strips are
# converted to fp8 expm1 by a casting accum DMA and join the DR stream.
# The sw-DGE ring sustains ~3.1us per [128,1024] accum conversion, capping
# the C class at ~6/group.
_ACT_EVENS = list(range(0, NJS, 2))
if USE_CSET:
    _CSET = [4, 10, 14, 20, 26, 30]  # 6 strips
    _BSET = [j for j in _ACT_EVENS if j not in _CSET]  # 10 strips
    _DVESET = list(range(1, NJS, 2))  # 16 strips
else:
    # v3 balance: 16 ACT / 16 DVE (both exp engines run ~100%; do not shift
    # strips between them)
    _BSET = list(_ACT_EVENS)  # 16 strips
    _CSET = []
    _DVESET = list(range(1, NJS, 2))  # 16 strips
_DRSET = sorted(_CSET + _DVESET)  # DoubleRow strips, paired in DSET order
NPAIR = len(_DRSET) // 2
assert len(_DRSET) % 2 == 0 and 0 in _BSET

AVLAG = 10  # AV queue depth (entries); covers the accum-DMA conversion lag
PTBUFS = 13  # bf16 pt pool depth
PTPBUFS = 10  # fp8 pair-pt pool depth
PO_DELAY = 6  # slots between epilogue emission and its po matmuls
NGROUPS = 8
# Only the LAST group broadcasts its recip on the PE: group 7's chain runs
# in the drain where the PE idles. Group 6's epilogue lands mid-stream of
# group 7, where the 2x853ns f32 bcast matmuls would cost busy PE time —
# its DMA bounce hides behind group 7's remaining ~14us instead, and its po
# gets +8 slots of delay so the bounce chain finishes before the po comes
# due (the v3 7.8us tail gap was the po stalling the in-order PE queue).
PE_BCAST_GROUPS = (7,)
PO_EXTRA_G6 = 8

_last_results = None  # test harness pokes this for exec_time_ns / profile


def _ap3(base2d, d1, n1, d2, n2):
    return bass.AP(
        tensor=base2d.tensor,
        offset=base2d.offset,
        ap=[list(base2d.ap[0]), [d1, n1], [d2, n2]],
    )


def _skip_ones_ap(base2d):
    """[P, 2, 32] AP over cols {1..32, 34..65} of a [P, >=66] slice starting
    at col 1 — the V-block minus the two ones columns."""
    return _ap3(base2d, 33, 2, 1, 32)


def _build():
    nc = bacc.Bacc(None, target_bir_lowering=False)
    xt_d = nc.declare_dram_parameter("xt", [C, N], BF16, isOutput=False)
    # packed: [Qh0|Qh1|Kh0|Kh1] 32 cols each; unpacked (v3): four zero-padded
    # 128-col blocks. Scale folded into Q either way.
    wqkT_d = nc.declare_dram_parameter(
        "wqkT", [C, 128 if USE_PACKED_PROJ else 512], BF16, isOutput=False
    )
    wvT_d = nc.declare_dram_parameter("wvT", [C, VBLK], BF16, isOutput=False)
    woT_d = nc.declare_dram_parameter("woT", [C, 256], BF16, isOutput=False)
    neg1_d = nc.declare_dram_parameter("neg1", [C, 1024], FP8, isOutput=False)
    out_d = nc.declare_dram_parameter("out", [C, N], F32, isOutput=True)
    recd = nc.dram_tensor("recd", [1, 1024], F32)  # reciprocal bounce
    corrd = nc.dram_tensor("corrd", [1, VBLK], F32)  # corr row bounce

    EXP = mybir.ActivationFunctionType.Exp
    IDENT = mybir.ActivationFunctionType.Identity

    with tile.TileContext(nc) as tc:
        with (
            tc.tile_pool(name="singles", bufs=1) as singles,
            tc.tile_pool(name="pts", bufs=PTBUFS) as pts,
            tc.tile_pool(name="ptps", bufs=PTPBUFS) as ptps,
            tc.tile_pool(name="simpool", bufs=3, space="PSUM") as simpool,
            tc.tile_pool(name="avpool", bufs=1, space="PSUM") as avpool,
        ):
            xt_s = singles.tile([C, N], BF16)
            wqkT_s = singles.tile([C, 128 if USE_PACKED_PROJ else 512], BF16)
            wvT_s = singles.tile([C, VBLK], BF16)
            woT_s = singles.tile([C, 256], BF16)
            if USE_PACKED_PROJ:
                # packed Q/K: rows 0-31 head0, rows 32-63 head1
                qkq = singles.tile([64, N], BF16)
                ktk = singles.tile([64, N], BF16)
            else:
                qt0 = singles.tile([C, N], BF16)  # head0 Q: rows 0-31
                kt0 = singles.tile([C, N], BF16)  # head0 K
                qt1 = singles.tile([C, N], BF16)  # head1 Q
                kt1 = singles.tile([C, N], BF16)  # head1 K
                qts = [qt0, qt1]
                kts = [kt0, kt1]
            vts = [
                singles.tile([C, 161], BF16, tag=f"vt{j}", name=f"vt{j}")
                for j in range(NJS)
            ]
            # fp8 paired V-blocks for DoubleRow: per (pair, head) a [128, 128]
            # tile, halves = [1|V(32)|0(31)] of the pair's two strips.
            vtp = [
                [
                    singles.tile([C, 128], FP8, tag=f"vtp{p}h{h}", name=f"vtp{p}h{h}")
                    for h in range(2)
                ]
                for p in range(NPAIR)
            ]
            onesb = singles.tile([C, 1], BF16)  # colsum stationary
            ones1r = singles.tile([1, 33], F32)  # PE-broadcast stationary
            corrS = singles.tile([C, 2], F32)  # per-head expm1 correction col
            avu = singles.tile([DH + 1, 1024], F32)
            avn = singles.tile([C, 2 * N], BF16)  # rows 33-127 zero
            rec = singles.tile([1, 1024], F32)  # 1/denom (bounce path)
            corrR = singles.tile([1, VBLK], F32)  # corr row staging
            bc = singles.tile([DH + 1, 1024], F32)  # broadcast recips
            outs = singles.tile([C, 1024], F32)  # po evacuation

            # prologue DMAs: small weights first on sync; first two x chunks
            # on the scalar queue so they land in parallel with the rest.
            nc.sync.dma_start(out=wqkT_s[:], in_=wqkT_d[:])
            nc.scalar.dma_start(out=xt_s[:, 0:IC], in_=xt_d[:, 0:IC])
            nc.scalar.dma_start(out=xt_s[:, IC : 2 * IC], in_=xt_d[:, IC : 2 * IC])
            for icd in range(2, NICH):
                nc.sync.dma_start(
                    out=xt_s[:, IC * icd : IC * (icd + 1)],
                    in_=xt_d[:, IC * icd : IC * (icd + 1)],
                )
            nc.gpsimd.dma_start(out=wvT_s[:], in_=wvT_d[:])
            nc.gpsimd.dma_start(out=woT_s[:], in_=woT_d[:])
            # GPSIMD prologue: avn junk rows (feed the final projection's
            # contraction), the colsum/broadcast ones. 32-partition chunks
            # (engine APs at partition>0 touch at most 32 partitions).
            nc.gpsimd.memset(avn[32:64, :], 0.0)
            nc.gpsimd.memset(avn[64:96, :], 0.0)
            nc.gpsimd.memset(avn[96:128, :], 0.0)
            nc.gpsimd.memset(onesb[:], 1.0)
            nc.gpsimd.memset(ones1r[:], 1.0)

            if USE_PACKED_PROJ:

                def qk_proj_one(icx):
                    # both heads' Q into ps[0:64, 0:512], K into [0:64, 512:]
                    ps = simpool.tile([128, 1024], F32, tag="sim")
                    for half in range(2):  # 0: Q-pack, 1: K-pack
                        nc.tensor.matmul(
                            ps[0:64, IC * half : IC * (half + 1)],
                            lhsT=wqkT_s[:, 64 * half : 64 * (half + 1)],
                            rhs=xt_s[:, IC * icx : IC * (icx + 1)],
                            start=True,
                            stop=True,
                        )
                    nc.scalar.copy(
                        qkq[:, IC * icx : IC * (icx + 1)], ps[0:64, 0:IC]
                    )
                    nc.vector.tensor_copy(
                        ktk[:, IC * icx : IC * (icx + 1)], ps[0:64, IC : 2 * IC]
                    )

                for icd in range(2):
                    qk_proj_one(icd)
            else:

                def qk_proj_one(h, icx):
                    # v3: zero-padded 128-col stationary blocks, one head at a
                    # time; q-copy on ACT, k-copy on DVE
                    ps = simpool.tile([128, 1024], F32, tag="sim")
                    for half in range(2):  # 0: Q, 1: K
                        cc = 2 * half + h
                        nc.tensor.matmul(
                            ps[:, IC * half : IC * (half + 1)],
                            lhsT=wqkT_s[:, 128 * cc : 128 * (cc + 1)],
                            rhs=xt_s[:, IC * icx : IC * (icx + 1)],
                            start=True,
                            stop=True,
                        )
                    nc.scalar.copy(
                        qts[h][:, IC * icx : IC * (icx + 1)], ps[:, 0:IC]
                    )
                    nc.vector.tensor_copy(
                        kts[h][:, IC * icx : IC * (icx + 1)], ps[:, IC : 2 * IC]
                    )

                for icd in range(2):
                    qk_proj_one(0, icd)

            def v_proj_one(jc):
                pv = simpool.tile([128, 1024], F32, tag="sim")
                nc.tensor.matmul(
                    pv[:, 0:VBLK],
                    lhsT=xt_s[:, JS * jc : JS * (jc + 1)],
                    rhs=wvT_s[:],
                    start=True,
                    stop=True,
                )
                nc.scalar.copy(
                    _skip_ones_ap(vts[jc][:, 1:2]), _skip_ones_ap(pv[:, 1:2])
                )
                # fp8 paired copies for DoubleRow strips (bf16 SBUF -> fp8,
                # on GPSIMD so neither exp engine pays for it)
                if jc in _DRSET:
                    k = _DRSET.index(jc)
                    p, halfi = k // 2, k % 2
                    for h in range(2):
                        nc.gpsimd.tensor_copy(
                            vtp[p][h][:, 64 * halfi + 1 : 64 * halfi + 33],
                            vts[jc][:, 33 * h + 1 : 33 * h + 33],
                        )

            VLEAD = 6
            for jc in range(VLEAD):
                v_proj_one(jc)

            groups = [(h, ip) for h in range(2) for ip in range(4)]
            gstate = [dict(ptp={}, npop=0) for _ in groups]
            av_q = []  # ('b', pt, js, gidx) | ('dr', ptp_tile, pair, gidx)
            po_q = []  # (due_slot, gidx)
            ep_q = []  # staged epilogue phases: (engine, thunk(slot))
            ENTRIES = len(_BSET) + NPAIR  # av_q entries per group

            def emit_corr():
                # corr[m] = sum over DR strips of colsum of the EXACT (bf16)
                # V-block, per head (restores the fp8 quantization-suppressed
                # "+1"): one ones-stationary accumulation over all DR strips
                # (single LoadStationary), then a DRAM bounce scatters the
                # [1,66] result row into corrS's per-partition bias layout.
                cp = simpool.tile([128, 1024], F32, tag="sim", name="corrP")
                for k, jc in enumerate(_DRSET):
                    nc.tensor.matmul(
                        cp[0:1, 0:VBLK],
                        lhsT=onesb[:, 0:1],
                        rhs=vts[jc][:, 0:VBLK],
                        start=(k == 0),
                        stop=(k == len(_DRSET) - 1),
                        skip_group_check=True,
                    )
                nc.vector.tensor_copy(corrR[0:1, 0:VBLK], cp[0:1, 0:VBLK])
                nc.sync.dma_start(out=corrd[0:1, 0:VBLK], in_=corrR[0:1, 0:VBLK])
                for h in range(2):
                    nc.sync.dma_start(
                        out=corrS[0:33, h : h + 1],
                        in_=corrd[0:1, 33 * h : 33 * h + 33],
                    )

            def emit_av(entry):
                kind, gidx = entry[0], entry[-1]
                st = gstate[gidx]
                h, ip = groups[gidx]
                if st["npop"] == 0:
                    st["ava"] = avpool.tile(
                        [C, IC], F32, tag="av_a", name=f"ava{gidx}"
                    )
                    st["avb"] = avpool.tile(
                        [C, IC], F32, tag="av_b", name=f"avb{gidx}"
                    )
                st["npop"] += 1
                first = st["npop"] == 1
                last = st["npop"] == ENTRIES
                if kind == "b":
                    _, apt, ajs, _ = entry
                    for t, av in enumerate((st["ava"], st["avb"])):
                        nc.tensor.matmul(
                            av[:],
                            lhsT=vts[ajs][:, 33 * h : 33 * h + 128],
                            rhs=apt[:, IC * t : IC * (t + 1)],
                            start=first,
                            stop=last,
                            skip_group_check=True,
                        )
                else:
                    _, ptile, pair, _ = entry
                    lhs3 = _ap3(vtp[pair][h][:, 0:1], 64, 2, 1, 64)
                    for t, av in enumerate((st["ava"], st["avb"])):
                        nc.tensor.matmul(
                            av[0:64, :],
                            lhsT=lhs3,
                            rhs=_ap3(ptile[:, IC * t : IC * t + 1], 1024, 2, 1, 512),
                            start=False,
                            stop=last,
                            perf_mode=DRMODE,
                            skip_group_check=True,
                        )
                return last

            def emit_epilogue(gidx, slot):
                h, ip = groups[gidx]
                st = gstate[gidx]
                hoff = N * h
                pe_bcast = gidx in PE_BCAST_GROUPS
                for t, av in enumerate((st["ava"], st["avb"])):
                    half = slice(512 * t, 512 * (t + 1))
                    # evacuate + add the expm1 correction column as bias;
                    # a-half on ACT, b-half on DVE (Pool cannot read PSUM)
                    if t == 0:
                        nc.scalar.activation(
                            avu[:, half],
                            av[0 : DH + 1, :],
                            IDENT,
                            bias=corrS[0 : DH + 1, h : h + 1],
                        )
                    else:
                        nc.vector.tensor_scalar_add(
                            avu[:, half],
                            av[0 : DH + 1, :],
                            corrS[0 : DH + 1, h : h + 1],
                        )
                    if not pe_bcast:
                        nc.vector.reciprocal_approx_fast(
                            rec[0:1, half], avu[0:1, half]
                        )
                if pe_bcast:
                    # tail groups: reciprocal -> bf16, broadcast across the 33
                    # partitions with a PE matmul into the just-retired av
                    # banks; skips the DRAM round-trip on the drain path.
                    nc.vector.reciprocal_approx_fast(
                        rec[0:1, 0:1024], avu[0:1, 0:1024]
                    )
                    pcbs = []
                    for t in range(2):
                        pcb = avpool.tile(
                            [C, IC], F32, tag=("av_a", "av_b")[t],
                            name=f"pcb{gidx}_{t}",
                        )
                        # f32 matmul (4 cycles/row) — the PE is idle in the
                        # tail, and this skips a serial f32->bf16 cast step
                        nc.tensor.matmul(
                            pcb[0:33, :],
                            lhsT=ones1r[0:1, 0:33],
                            rhs=rec[0:1, 512 * t : 512 * (t + 1)],
                            start=True,
                            stop=True,
                            skip_group_check=True,
                        )
                        pcbs.append(pcb)
                    for t in range(2):
                        ic = 2 * ip + t
                        sl = slice(hoff + IC * ic, hoff + IC * (ic + 1))
                        half = slice(512 * t, 512 * (t + 1))
                        nc.vector.tensor_mul(
                            avn[0 : DH + 1, sl], avu[:, half], pcbs[t][0:33, :]
                        )
                else:
                    # one bounce round-trip for both chunks
                    nc.sync.dma_start(out=recd[0:1, 0:1024], in_=rec[0:1, 0:1024])
                    dsl = recd[0:1, 0:1024]
                    nc.sync.dma_start(
                        out=bc[:, 0:1024],
                        in_=bass.AP(
                            tensor=dsl.tensor,
                            offset=dsl.offset,
                            ap=[[0, DH + 1]] + list(dsl.ap[1:]),
                        ),
                    )
                    # normalize muls both on Pool (SBUF-only operands; with
                    # the C-class off there are no accum issues to block, and
                    # this removes 0.7us from DVE's group-boundary burst that
                    # was starving the 3-buffer sim pool). PO_DELAY covers
                    # Pool's serial ~3.3us before the po reads avn.
                    for t in range(2):
                        ic = 2 * ip + t
                        sl = slice(hoff + IC * ic, hoff + IC * (ic + 1))
                        half = slice(512 * t, 512 * (t + 1))
                        nc.gpsimd.tensor_mul(
                            avn[0 : DH + 1, sl], avu[:, half], bc[:, half]
                        )
                if h == 1:
                    extra = PO_EXTRA_G6 if gidx == 6 else 0
                    po_q.append((slot + PO_DELAY + extra, gidx))

            def stage_epilogue(gidx):
                # Spread the epilogue over one piece per slot, parity-matched
                # so each piece lands on the slot where its engine has no exp
                # (ACT exps run on even slots, DVE exps on odd): the 0.7-1.1us
                # pieces then fit the exp engines' per-slot headroom instead
                # of forming a ~2.5us burst that starves the 3-buffer sim
                # pool and stalls the PE at every group boundary.
                h, ip = groups[gidx]
                st = gstate[gidx]
                hoff = N * h

                def p0(s):  # ACT: evacuate a-half + corr bias
                    nc.scalar.activation(
                        avu[:, 0:512],
                        st["ava"][0 : DH + 1, :],
                        IDENT,
                        bias=corrS[0 : DH + 1, h : h + 1],
                    )

                def p1(s):  # DVE: evacuate b-half + corr bias
                    nc.vector.tensor_scalar_add(
                        avu[:, 512:1024],
                        st["avb"][0 : DH + 1, :],
                        corrS[0 : DH + 1, h : h + 1],
                    )

                def p2(s):  # DVE: reciprocal of both halves' denominators
                    nc.vector.reciprocal_approx_fast(
                        rec[0:1, 0:1024], avu[0:1, 0:1024]
                    )

                def p3(s):  # sync: DRAM bounce broadcast
                    nc.sync.dma_start(out=recd[0:1, 0:1024], in_=rec[0:1, 0:1024])
                    dsl = recd[0:1, 0:1024]
                    nc.sync.dma_start(
                        out=bc[:, 0:1024],
                        in_=bass.AP(
                            tensor=dsl.tensor,
                            offset=dsl.offset,
                            ap=[[0, DH + 1]] + list(dsl.ap[1:]),
                        ),
                    )

                def p4(s):  # Pool: normalize muls; then schedule po
                    for t in range(2):
                        ic = 2 * ip + t
                        sl = slice(hoff + IC * ic, hoff + IC * (ic + 1))
                        half = slice(512 * t, 512 * (t + 1))
                        nc.gpsimd.tensor_mul(
                            avn[0 : DH + 1, sl], avu[:, half], bc[:, half]
                        )
                    if h == 1:
                        # all staged h==1 groups: the bounce+mul chain takes
                        # ~11 slots; without the extra delay the po stalls
                        # the in-order PE queue at each boundary (same
                        # mechanism as the group-6 fix that bought 5.3us)
                        extra = PO_EXTRA_G6
                        po_q.append((s + PO_DELAY + extra, gidx))

                ep_q.extend(
                    [("act", p0), ("dve", p1), ("dve", p2), ("any", p3), ("any", p4)]
                )

            def pump_ep_q(slot):
                # run at most one staged phase per slot, on the right parity
                if not ep_q:
                    return
                eng, fn = ep_q[0]
                if (
                    eng == "any"
                    or (eng == "act" and slot % 2 == 1)
                    or (eng == "dve" and slot % 2 == 0)
                ):
                    ep_q.pop(0)
                    fn(slot)

            def emit_po(gidx, last=False):
                h, ip = groups[gidx]
                po = simpool.tile([128, 1024], F32, tag="sim", name=f"po{gidx}")
                for t in range(2):
                    ic = 2 * ip + t
                    nc.tensor.matmul(
                        po[:, 512 * t : 512 * t + IC],
                        lhsT=woT_s[:, 0:128],
                        rhs=avn[:, IC * ic : IC * (ic + 1)],
                        start=True,
                        stop=False,
                    )
                    nc.tensor.matmul(
                        po[:, 512 * t : 512 * t + IC],
                        lhsT=woT_s[:, 128:256],
                        rhs=avn[:, N + IC * ic : N + IC * (ic + 1)],
                        start=False,
                        stop=True,
                    )
                # bias is folded into the projection (avn row 0 = denom*recip
                # = 1, woT row 0 of block 0 is bo); evacuate the two chunks on
                # different engines; the last po also fans its DMAs across the
                # sync+scalar queues so the drain doesn't serialize.
                for t in range(2):
                    ic = 2 * ip + t
                    half = slice(512 * t, 512 * (t + 1))
                    eng = nc.scalar.copy if t == 0 else nc.vector.tensor_copy
                    eng(outs[:, half], po[:, half])
                    dq = nc.scalar if (last and t == 1) else nc.sync
                    dq.dma_start(
                        out=out_d[:, IC * ic : IC * (ic + 1)],
                        in_=outs[:, half],
                    )

            slot = 0
            for gidx, (h, ip) in enumerate(groups):
                ica, icb = 2 * ip, 2 * ip + 1
                for js in range(NJS):
                    while po_q and po_q[0][0] <= slot:
                        emit_po(po_q.pop(0)[1])
                    pump_ep_q(slot)
                    if gidx == 0:
                        nc.gpsimd.memset(
                            _ap3(vts[js][:, 0:1], 33, 2, 1, 1), 1.0
                        )
                        if js < NPAIR:
                            for hh in range(2):
                                nc.gpsimd.memset(
                                    _ap3(vtp[js][hh][:, 0:1], 64, 2, 1, 1), 1.0
                                )
                    if USE_PACKED_PROJ:
                        if h == 0 and ip == 0 and js < 12 and js % 2 == 0:
                            qk_proj_one(2 + js // 2)
                    else:
                        if h == 0 and ip == 0 and js < NICH - 2:
                            qk_proj_one(0, js + 2)
                        if h == 0 and ip in (1, 2) and js % 8 == 0:
                            qk_proj_one(1, 4 * (ip - 1) + js // 8)
                    if h == 0 and ip == 0 and js < NJS - VLEAD:
                        v_proj_one(js + VLEAD)
                    if h == 0 and ip == 1 and js == 0:
                        emit_corr()
                    sim = simpool.tile([128, 1024], F32, tag="sim")
                    for t, icx in enumerate((ica, icb)):
                        if USE_PACKED_PROJ:
                            s_lhs = ktk[32 * h : 32 * h + 32, JS * js : JS * (js + 1)]
                            s_rhs = qkq[32 * h : 32 * h + 32, IC * icx : IC * (icx + 1)]
                        else:
                            s_lhs = kts[h][:, JS * js : JS * (js + 1)]
                            s_rhs = qts[h][:, IC * icx : IC * (icx + 1)]
                        nc.tensor.matmul(
                            sim[:, IC * t : IC * (t + 1)],
                            lhsT=s_lhs,
                            rhs=s_rhs,
                            start=True,
                            stop=True,
                        )
                    if js in _BSET:
                        pt = pts.tile([128, 1024], BF16, tag="pt")
                        nc.scalar.activation(pt[:], sim[:], EXP)
                        av_q.append(("b", pt, js, gidx))
                    else:
                        k = _DRSET.index(js)
                        pair, halfi = k // 2, k % 2
                        st = gstate[gidx]
                        if halfi == 0:
                            st["ptp"][pair] = ptps.tile(
                                [128, 2048], FP8, tag="ptp",
                                name=f"ptp{gidx}_{pair}",
                            )
                            # prefill any C-class halves of this pair with
                            # -1.0 so the casting accum DMA lands expm1
                            for hi in range(2):
                                if _DRSET[2 * pair + hi] in _CSET:
                                    nc.sync.dma_start(
                                        out=st["ptp"][pair][
                                            :, 1024 * hi : 1024 * (hi + 1)
                                        ],
                                        in_=neg1_d[:],
                                    )
                        ptile = st["ptp"][pair]
                        dst = ptile[:, 1024 * halfi : 1024 * (halfi + 1)]
                        if js in _CSET:
                            pt = pts.tile([128, 1024], BF16, tag="pt")
                            nc.scalar.activation(pt[:], sim[:], EXP)
                            nc.gpsimd.dma_start(out=dst, in_=pt[:], accum_op=ADD)
                        else:
                            nc.vector._custom_dve(
                                _EXPM1_OP, out=dst, in0=sim[:],
                                s0=_EXPC2, s1=_EXPC3, imm2=_EXPC4,
                            )
                        if halfi == 1:
                            av_q.append(("dr", ptile, pair, gidx))
                    if len(av_q) > AVLAG:
                        entry = av_q.pop(0)
                        if emit_av(entry):
                            g = entry[-1]
                            if g in PE_BCAST_GROUPS:
                                emit_epilogue(g, slot)
                            else:
                                stage_epilogue(g)
                    slot += 1

            while av_q:
                pump_ep_q(slot)
                entry = av_q.pop(0)
                if emit_av(entry):
                    g = entry[-1]
                    if g in PE_BCAST_GROUPS:
                        emit_epilogue(g, slot)
                    else:
                        stage_epilogue(g)
                slot += 1
            while ep_q:
                ep_q.pop(0)[1](slot)
                slot += 1
            while po_q:
                emit_po(po_q.pop(0)[1], last=(len(po_q) == 0))
    nc.finalize()
    return nc


_nc_cache = None


def _get_nc():
    global _nc_cache
    if _nc_cache is None:
        _nc_cache = _build()
    return _nc_cache


def make_in_maps(x, wq, wk, wv, wo, bo):
    b = 4
    xt = np.asarray(x, np.float32).reshape(b, C, N)
    wq = np.asarray(wq, np.float32)
    wk = np.asarray(wk, np.float32)
    wv = np.asarray(wv, np.float32)
    wo = np.asarray(wo, np.float32)
    bo = np.asarray(bo, np.float32)
    scale = DH ** (-0.5)

    def bf(a):
        return np.ascontiguousarray(a.astype(ml_dtypes.bfloat16))

    neg1 = np.full((C, 1024), -1.0, dtype=ml_dtypes.float8_e4m3fn)
    in_maps = []
    for core in range(8):
        bi, hp = core // 2, core % 2
        wq2 = wq[64 * hp : 64 * hp + 64] * scale
        wk2 = wk[64 * hp : 64 * hp + 64]
        wv2 = wv[64 * hp : 64 * hp + 64]
        if USE_PACKED_PROJ:
            # packed stationary: [Qh0|Qh1|Kh0|Kh1], 32 cols each
            wqkT = np.zeros((C, 128), np.float32)
            wqkT[:, 0:32] = wq2.T[:, 0:32]
            wqkT[:, 32:64] = wq2.T[:, 32:64]
            wqkT[:, 64:96] = wk2.T[:, 0:32]
            wqkT[:, 96:128] = wk2.T[:, 32:64]
        else:
            # v3: four zero-padded 128-col blocks [Qh0|Qh1|Kh0|Kh1]
            wqkT = np.zeros((C, 512), np.float32)
            wqkT[:, 0:32] = wq2.T[:, 0:32]
            wqkT[:, 128:160] = wq2.T[:, 32:64]
            wqkT[:, 256:288] = wk2.T[:, 0:32]
            wqkT[:, 384:416] = wk2.T[:, 32:64]
        wvT = np.zeros((C, VBLK), np.float32)  # cols 0,33 stay 0 (ones in SBUF)
        wvT[:, 1:33] = wv2.T[:, 0:32]
        wvT[:, 34:66] = wv2.T[:, 32:64]
        woT = np.zeros((C, 256), np.float32)
        woT[1:33, 0:128] = wo[:, 64 * hp : 64 * hp + 32].T
        woT[1:33, 128:256] = wo[:, 64 * hp + 32 : 64 * hp + 64].T
        if hp == 0:
            woT[0, 0:128] = bo  # bias rides avn row 0 (= denom/denom = 1)
        in_maps.append(
            {
                "xt": bf(xt[bi]),
                "wqkT": bf(wqkT),
                "wvT": bf(wvT),
                "woT": bf(woT),
                "neg1": neg1,
            }
        )
    return in_maps


def kernel(x, wq, wk, wv, wo, bo):
    global _last_results
    in_maps = make_in_maps(x, wq, wk, wv, wo, bo)
    nc = _get_nc()
    res = run_bass_kernel_spmd(nc, in_maps, core_ids=list(range(8)))
    _last_results = res
    outs = res.results
    out = np.zeros((4, C, N), np.float32)
    for bi in range(4):
        out[bi] = np.asarray(outs[2 * bi]["out"], np.float32) + np.asarray(
            outs[2 * bi + 1]["out"], np.float32
        )
    return out.reshape(4, C, 64, 64)


# revision 56
# speedup vs baseline: 1.0037x; 1.0037x over previous
"""Trainium2 8-core attention kernel for nn_Attention_14104672600564.

Problem: x[4,128,64,64] f32; wq/wk/wv/wo [128,128]; bo[128].
  per (b,h): sim = (wq x)^T (wk x) * d^-.5 ; attn = softmax(sim) ; out_h = attn @ (wv x)^T
  out = wo @ concat_h(out_h^T) + bo

Sharding: 16 independent (batch, head) attention problems -> 8 cores; each core
gets one batch and one head-pair; the host unshards by summing the two cores of
each batch.

v4.2 design, 253.0us (v4/v4.1 with accum-DMA C-class: 294/286us — worse;
v3: 256.2us; v2: 250us bf16; v1: 284us):
  - Strip classes per 32 j-strips, alternating ACT/DVE by parity so neither
    exp engine sees bursts: B (16 even js, ACT exp -> bf16 pt -> bf16 AV),
    D (16 odd js, DVE custom poly expm1 -> fp8 -> DoubleRow AV). Both exp
    engines run ~100% in steady state; the equilibrium wall is their
    throughput, so nothing extra may ride them mid-stream.
  - The C-class experiment (gpsimd casting accum DMA adds ACT's bf16 exp
    onto a -1-prefilled fp8 tile -> fp8 expm1 -> DR AV) is HW-correct and
    numerically ideal, but its multi-queue chain (ACT -> Pool issue ~1us ->
    sw ring ~3.1us -> DR AV) jitters the saturated engines; PE gaps grew
    2x more than the ~10us of PE work it saved. Disabled via USE_CSET.
    Partition-offset matmul inputs (base>0) pass BIR verify but CRASH the
    runtime — packed Q/K projection disabled via USE_PACKED_PROJ.
  - corr (the expm1 "+1" restoration column) via a single ones-stationary
    accumulation: out[0,m] = sum_j vts[j,m] over all DR strips (one
    LoadStationary, 66-col matmuls), then a DRAM bounce scatters the
    [1,66] row into the per-partition corrS[33,2] bias layout.
  - Tail (saves ~6us): the last two groups broadcast their reciprocal via a
    contract-1 f32 PE matmul (ones[1,33] x rec[1,512]) into the retired av
    PSUM banks instead of the DRAM round-trip — the PE is idle in the drain
    and this skips two DMA+semaphore hops; their normalize muls run on DVE
    reading the PSUM broadcast; the final po fans its two output DMAs
    across the sync+scalar queues.
  - Prologue: wqkT first on sync, x chunks 0-1 on the scalar queue so the
    first projections and the rest of x land in parallel.
  - Flat pipeline, AVLAG queue, PO_DELAY borrowed sim slots as in v3.
"""

import sys

sys.path.insert(0, "/opt/trn_rl_repo")

import numpy as np
import ml_dtypes

import concourse.bass as bass
import concourse.bacc as bacc
import concourse.tile as tile
from concourse import mybir
import concourse.bass_utils as _bass_utils
from concourse.bass_utils import run_bass_kernel_spmd

# ---- custom DVE exp ops (degree-4 polynomial, |x| <= ~0.5) -----------------
import concourse.dve_ops as _dvo
from concourse.dve_ops import DveOp as _DveOp
from concourse.dve_spec import (
    Spec as _Spec,
    Src0 as _Src0,
    C0 as _C0,
    C1 as _C1,
    C2 as _C2,
    One as _One,
    lower as _dve_lower,
)
from concourse.dve_uop import DveOpSpec as _DveOpSpec

# fit of exp(x) on [-0.45, 0.45] with c0=c1=1 fixed:
# exp(x) ~ 1 + x*(1 + x*(c2 + x*(c3 + x*c4))); max rel err 3.5e-5
_EXPC2, _EXPC3, _EXPC4 = 0.50019703, 0.16796468, 0.04051121


def _register_op(name, body, reference):
    for op in _dvo.OPS:
        if op.name == name:
            return op
    spec = _Spec(body=body, reference=reference)
    row = _dvo._CUSTOM_DVE_ROW_BASE + len(_dvo.OPS)
    shas = {}
    for ver in ("v3", "v4"):
        uops = _dve_lower(spec, ver=ver)
        shas[ver] = _DveOpSpec(
            name=name, opcode=row, uops=uops, rd1_en=False
        ).sha(ver)
    op = _DveOp(name, spec, subdim=False, uops_sha=shas)
    _dvo.OPS.append(op)
    _dvo._SUB_OPCODE_FOR_NAME[name] = row
    return op


def _register_exp_ops():
    t = _Src0 * _C2 + _C1
    t = t * _Src0 + _C0
    t = t * _Src0 + _One
    exp_op = _register_op(
        "EXP_POLY4_ANT",
        t * _Src0 + _One,
        lambda in0, in1, s0, s1, imm2: (
            (((imm2 * in0 + s1) * in0 + s0) * in0 + 1.0) * in0 + 1.0
        ),
    )
    em1_op = _register_op(
        "EXPM1_POLY4_ANT",
        t * _Src0,
        lambda in0, in1, s0, s1, imm2: (
            (((imm2 * in0 + s1) * in0 + s0) * in0 + 1.0) * in0
        ),
    )
    return exp_op, em1_op


_EXP_OP, _EXPM1_OP = _register_exp_ops()


BF16 = mybir.dt.bfloat16
F32 = mybir.dt.float32
FP8 = mybir.dt.float8e4
DRMODE = mybir.MatmulPerfMode.DoubleRow
ADD = mybir.AluOpType.add

HEADS = 4
DH = 32  # dim per head
C = 128  # channels
N = 4096  # tokens (64*64)
IC = 512  # i-chunk
NICH = N // IC  # 8
JS = 128  # j-strip
NJS = N // JS  # 32
VBLK = 2 * (DH + 1)  # 66: [1|Vh0|1|Vh1] per j-strip

# feature toggles (fallbacks are the v3-proven paths)
# The accum-DMA C-class is HW-validated but its multi-queue conversion chain
# (ACT -> Pool issue -> sw ring -> DR AV) adds enough jitter to the ~100%%-
# loaded exp engines that PE gaps grow more than the ~10us of PE work it
# saves (294/286us vs 256us benches). Disabled.
USE_CSET = False
# partition-offset matmul inputs crash the runtime (probe2 B): keep the
# v3-style zero-padded per-head projection.
USE_PACKED_PROJ = False
USE_PE_BCAST = True  # tail groups broadcast recip via PE instead of DMA bounce
USE_CORR_ONES = True  # corr via ones-stationary matmul + bounce transpose

# strip classes: even js -> ACT exp, odd js -> DVE expm1 (alternation keeps
# both exp engines fed each slot). Within the ACT evens, B-class strips stay
# bf16 (their AV runs bf16, carrying start=True on js0); C-class strips are
# converted to fp8 expm1 by a casting accum DMA and join the DR stream.
# The sw-DGE ring sustains ~3.1us per [128,1024] accum conversion, capping
# the C class at ~6/group.
_ACT_EVENS = list(range(0, NJS, 2))
if USE_CSET:
    _CSET = [4, 10, 14, 20, 26, 30]  # 6 strips
    _BSET = [j for j in _ACT_EVENS if j not in _CSET]  # 10 strips
    _DVESET = list(range(1, NJS, 2))  # 16 strips
else:
    # v3 balance: 16 ACT / 16 DVE (both exp engines run ~100%; do not shift
    # strips between them)
    _BSET = list(_ACT_EVENS)  # 16 strips
    _CSET = []
    _DVESET = list(range(1, NJS, 2))  # 16 strips
_DRSET = sorted(_CSET + _DVESET)  # DoubleRow strips, paired in DSET order
NPAIR = len(_DRSET) // 2
assert len(_DRSET) % 2 == 0 and 0 in _BSET

AVLAG = 10  # AV queue depth (entries); covers the accum-DMA conversion lag
PTBUFS = 13  # bf16 pt pool depth
PTPBUFS = 10  # fp8 pair-pt pool depth
PO_DELAY = 6  # slots between epilogue emission and its po matmuls
NGROUPS = 8
# Only the LAST group broadcasts its recip on the PE: group 7's chain runs
# in the drain where the PE idles. Group 6's epilogue lands mid-stream of
# group 7, where the 2x853ns f32 bcast matmuls would cost busy PE time —
# its DMA bounce hides behind group 7's remaining ~14us instead, and its po
# gets +8 slots of delay so the bounce chain finishes before the po comes
# due (the v3 7.8us tail gap was the po stalling the in-order PE queue).
PE_BCAST_GROUPS = (7,)
PO_EXTRA_G6 = 8

_last_results = None  # test harness pokes this for exec_time_ns / profile


def _ap3(base2d, d1, n1, d2, n2):
    return bass.AP(
        tensor=base2d.tensor,
        offset=base2d.offset,
        ap=[list(base2d.ap[0]), [d1, n1], [d2, n2]],
    )


def _skip_ones_ap(base2d):
    """[P, 2, 32] AP over cols {1..32, 34..65} of a [P, >=66] slice starting
    at col 1 — the V-block minus the two ones columns."""
    return _ap3(base2d, 33, 2, 1, 32)


def _build():
    nc = bacc.Bacc(None, target_bir_lowering=False)
    xt_d = nc.declare_dram_parameter("xt", [C, N], BF16, isOutput=False)
    # packed: [Qh0|Qh1|Kh0|Kh1] 32 cols each; unpacked (v3): four zero-padded
    # 128-col blocks. Scale folded into Q either way.
    wqkT_d = nc.declare_dram_parameter(
        "wqkT", [C, 128 if USE_PACKED_PROJ else 512], BF16, isOutput=False
    )
    wvT_d = nc.declare_dram_parameter("wvT", [C, VBLK], BF16, isOutput=False)
    woT_d = nc.declare_dram_parameter("woT", [C, 256], BF16, isOutput=False)
    neg1_d = nc.declare_dram_parameter("neg1", [C, 1024], FP8, isOutput=False)
    out_d = nc.declare_dram_parameter("out", [C, N], F32, isOutput=True)
    recd = nc.dram_tensor("recd", [1, 1024], F32)  # reciprocal bounce
    corrd = nc.dram_tensor("corrd", [1, VBLK], F32)  # corr row bounce

    EXP = mybir.ActivationFunctionType.Exp
    IDENT = mybir.ActivationFunctionType.Identity

    with tile.TileContext(nc) as tc:
        with (
            tc.tile_pool(name="singles", bufs=1) as singles,
            tc.tile_pool(name="pts", bufs=PTBUFS) as pts,
            tc.tile_pool(name="ptps", bufs=PTPBUFS) as ptps,
            tc.tile_pool(name="simpool", bufs=3, space="PSUM") as simpool,
            tc.tile_pool(name="avpool", bufs=1, space="PSUM") as avpool,
        ):
            xt_s = singles.tile([C, N], BF16)
            wqkT_s = singles.tile([C, 128 if USE_PACKED_PROJ else 512], BF16)
            wvT_s = singles.tile([C, VBLK], BF16)
            woT_s = singles.tile([C, 256], BF16)
            if USE_PACKED_PROJ:
                # packed Q/K: rows 0-31 head0, rows 32-63 head1
                qkq = singles.tile([64, N], BF16)
                ktk = singles.tile([64, N], BF16)
            else:
                qt0 = singles.tile([C, N], BF16)  # head0 Q: rows 0-31
                kt0 = singles.tile([C, N], BF16)  # head0 K
                qt1 = singles.tile([C, N], BF16)  # head1 Q
                kt1 = singles.tile([C, N], BF16)  # head1 K
                qts = [qt0, qt1]
                kts = [kt0, kt1]
            vts = [
                singles.tile([C, 161], BF16, tag=f"vt{j}", name=f"vt{j}")
                for j in range(NJS)
            ]
            # fp8 paired V-blocks for DoubleRow: per (pair, head) a [128, 128]
            # tile, halves = [1|V(32)|0(31)] of the pair's two strips.
            vtp = [
                [
                    singles.tile([C, 128], FP8, tag=f"vtp{p}h{h}", name=f"vtp{p}h{h}")
                    for h in range(2)
                ]
                for p in range(NPAIR)
            ]
            onesb = singles.tile([C, 1], BF16)  # colsum stationary
            ones1r = singles.tile([1, 33], F32)  # PE-broadcast stationary
            corrS = singles.tile([C, 2], F32)  # per-head expm1 correction col
            avu = singles.tile([DH + 1, 1024], F32)
            avn = singles.tile([C, 2 * N], BF16)  # rows 33-127 zero
            rec = singles.tile([1, 1024], F32)  # 1/denom (bounce path)
            corrR = singles.tile([1, VBLK], F32)  # corr row staging
            bc = singles.tile([DH + 1, 1024], F32)  # broadcast recips
            outs = singles.tile([C, 1024], F32)  # po evacuation

            # prologue DMAs: small weights first on sync; first two x chunks
            # on the scalar queue so they land in parallel with the rest.
            nc.sync.dma_start(out=wqkT_s[:], in_=wqkT_d[:])
            nc.scalar.dma_start(out=xt_s[:, 0:IC], in_=xt_d[:, 0:IC])
            nc.scalar.dma_start(out=xt_s[:, IC : 2 * IC], in_=xt_d[:, IC : 2 * IC])
            for icd in range(2, NICH):
                nc.sync.dma_start(
                    out=xt_s[:, IC * icd : IC * (icd + 1)],
                    in_=xt_d[:, IC * icd : IC * (icd + 1)],
                )
            nc.gpsimd.dma_start(out=wvT_s[:], in_=wvT_d[:])
            nc.gpsimd.dma_start(out=woT_s[:], in_=woT_d[:])
            # GPSIMD prologue: avn junk rows (feed the final projection's
            # contraction), the colsum/broadcast ones. 32-partition chunks
            # (engine APs at partition>0 touch at most 32 partitions).
            nc.gpsimd.memset(avn[32:64, :], 0.0)
            nc.gpsimd.memset(avn[64:96, :], 0.0)
            nc.gpsimd.memset(avn[96:128, :], 0.0)
            nc.gpsimd.memset(onesb[:], 1.0)
            nc.gpsimd.memset(ones1r[:], 1.0)

            if USE_PACKED_PROJ:

                def qk_proj_one(icx):
                    # both heads' Q into ps[0:64, 0:512], K into [0:64, 512:]
                    ps = simpool.tile([128, 1024], F32, tag="sim")
                    for half in range(2):  # 0: Q-pack, 1: K-pack
                        nc.tensor.matmul(
                            ps[0:64, IC * half : IC * (half + 1)],
                            lhsT=wqkT_s[:, 64 * half : 64 * (half + 1)],
                            rhs=xt_s[:, IC * icx : IC * (icx + 1)],
                            start=True,
                            stop=True,
                        )
                    nc.scalar.copy(
                        qkq[:, IC * icx : IC * (icx + 1)], ps[0:64, 0:IC]
                    )
                    nc.vector.tensor_copy(
                        ktk[:, IC * icx : IC * (icx + 1)], ps[0:64, IC : 2 * IC]
                    )

                for icd in range(2):
                    qk_proj_one(icd)
            else:

                def qk_proj_one(h, icx):
                    # v3: zero-padded 128-col stationary blocks, one head at a
                    # time; q-copy on ACT, k-copy on DVE
                    ps = simpool.tile([128, 1024], F32, tag="sim")
                    for half in range(2):  # 0: Q, 1: K
                        cc = 2 * half + h
                        nc.tensor.matmul(
                            ps[:, IC * half : IC * (half + 1)],
                            lhsT=wqkT_s[:, 128 * cc : 128 * (cc + 1)],
                            rhs=xt_s[:, IC * icx : IC * (icx + 1)],
                            start=True,
                            stop=True,
                        )
                    nc.scalar.copy(
                        qts[h][:, IC * icx : IC * (icx + 1)], ps[:, 0:IC]
                    )
                    nc.vector.tensor_copy(
                        kts[h][:, IC * icx : IC * (icx + 1)], ps[:, IC : 2 * IC]
                    )

                for icd in range(2):
                    qk_proj_one(0, icd)

            def v_proj_one(jc):
                pv = simpool.tile([128, 1024], F32, tag="sim")
                nc.tensor.matmul(
                    pv[:, 0:VBLK],
                    lhsT=xt_s[:, JS * jc : JS * (jc + 1)],
                    rhs=wvT_s[:],
                    start=True,
                    stop=True,
                )
                nc.scalar.copy(
                    _skip_ones_ap(vts[jc][:, 1:2]), _skip_ones_ap(pv[:, 1:2])
                )
                # fp8 paired copies for DoubleRow strips (bf16 SBUF -> fp8,
                # on GPSIMD so neither exp engine pays for it)
                if jc in _DRSET:
                    k = _DRSET.index(jc)
                    p, halfi = k // 2, k % 2
                    for h in range(2):
                        nc.gpsimd.tensor_copy(
                            vtp[p][h][:, 64 * halfi + 1 : 64 * halfi + 33],
                            vts[jc][:, 33 * h + 1 : 33 * h + 33],
                        )

            VLEAD = 6
            for jc in range(VLEAD):
                v_proj_one(jc)

            groups = [(h, ip) for h in range(2) for ip in range(4)]
            gstate = [dict(ptp={}, npop=0) for _ in groups]
            av_q = []  # ('b', pt, js, gidx) | ('dr', ptp_tile, pair, gidx)
            po_q = []  # (due_slot, gidx)
            ep_q = []  # staged epilogue phases: (engine, thunk(slot))
            ENTRIES = len(_BSET) + NPAIR  # av_q entries per group

            def emit_corr():
                # corr[m] = sum over DR strips of colsum of the EXACT (bf16)
                # V-block, per head (restores the fp8 quantization-suppressed
                # "+1"): one ones-stationary accumulation over all DR strips
                # (single LoadStationary), then a DRAM bounce scatters the
                # [1,66] result row into corrS's per-partition bias layout.
                cp = simpool.tile([128, 1024], F32, tag="sim", name="corrP")
                for k, jc in enumerate(_DRSET):
                    nc.tensor.matmul(
                        cp[0:1, 0:VBLK],
                        lhsT=onesb[:, 0:1],
                        rhs=vts[jc][:, 0:VBLK],
                        start=(k == 0),
                        stop=(k == len(_DRSET) - 1),
                        skip_group_check=True,
                    )
                nc.vector.tensor_copy(corrR[0:1, 0:VBLK], cp[0:1, 0:VBLK])
                nc.sync.dma_start(out=corrd[0:1, 0:VBLK], in_=corrR[0:1, 0:VBLK])
                for h in range(2):
                    nc.sync.dma_start(
                        out=corrS[0:33, h : h + 1],
                        in_=corrd[0:1, 33 * h : 33 * h + 33],
                    )

            def emit_av(entry):
                kind, gidx = entry[0], entry[-1]
                st = gstate[gidx]
                h, ip = groups[gidx]
                if st["npop"] == 0:
                    st["ava"] = avpool.tile(
                        [C, IC], F32, tag="av_a", name=f"ava{gidx}"
                    )
                    st["avb"] = avpool.tile(
                        [C, IC], F32, tag="av_b", name=f"avb{gidx}"
                    )
                st["npop"] += 1
                first = st["npop"] == 1
                last = st["npop"] == ENTRIES
                if kind == "b":
                    _, apt, ajs, _ = entry
                    for t, av in enumerate((st["ava"], st["avb"])):
                        nc.tensor.matmul(
                            av[:],
                            lhsT=vts[ajs][:, 33 * h : 33 * h + 128],
                            rhs=apt[:, IC * t : IC * (t + 1)],
                            start=first,
                            stop=last,
                            skip_group_check=True,
                        )
                else:
                    _, ptile, pair, _ = entry
                    lhs3 = _ap3(vtp[pair][h][:, 0:1], 64, 2, 1, 64)
                    for t, av in enumerate((st["ava"], st["avb"])):
                        nc.tensor.matmul(
                            av[0:64, :],
                            lhsT=lhs3,
                            rhs=_ap3(ptile[:, IC * t : IC * t + 1], 1024, 2, 1, 512),
                            start=False,
                            stop=last,
                            perf_mode=DRMODE,
                            skip_group_check=True,
                        )
                return last

            def emit_epilogue(gidx, slot):
                h, ip = groups[gidx]
                st = gstate[gidx]
                hoff = N * h
                pe_bcast = gidx in PE_BCAST_GROUPS
                for t, av in enumerate((st["ava"], st["avb"])):
                    half = slice(512 * t, 512 * (t + 1))
                    # evacuate + add the expm1 correction column as bias;
                    # a-half on ACT, b-half on DVE (Pool cannot read PSUM)
                    if t == 0:
                        nc.scalar.activation(
                            avu[:, half],
                            av[0 : DH + 1, :],
                            IDENT,
                            bias=corrS[0 : DH + 1, h : h + 1],
                        )
                    else:
                        nc.vector.tensor_scalar_add(
                            avu[:, half],
                            av[0 : DH + 1, :],
                            corrS[0 : DH + 1, h : h + 1],
                        )
                    if not pe_bcast:
                        nc.vector.reciprocal_approx_fast(
                            rec[0:1, half], avu[0:1, half]
                        )
                if pe_bcast:
                    # tail groups: reciprocal -> bf16, broadcast across the 33
                    # partitions with a PE matmul into the just-retired av
                    # banks; skips the DRAM round-trip on the drain path.
                    nc.vector.reciprocal_approx_fast(
                        rec[0:1, 0:1024], avu[0:1, 0:1024]
                    )
                    pcbs = []
                    for t in range(2):
                        pcb = avpool.tile(
                            [C, IC], F32, tag=("av_a", "av_b")[t],
                            name=f"pcb{gidx}_{t}",
                        )
                        # f32 matmul (4 cycles/row) — the PE is idle in the
                        # tail, and this skips a serial f32->bf16 cast step
                        nc.tensor.matmul(
                            pcb[0:33, :],
                            lhsT=ones1r[0:1, 0:33],
                            rhs=rec[0:1, 512 * t : 512 * (t + 1)],
                            start=True,
                            stop=True,
                            skip_group_check=True,
                        )
                        pcbs.append(pcb)
                    for t in range(2):
                        ic = 2 * ip + t
                        sl = slice(hoff + IC * ic, hoff + IC * (ic + 1))
                        half = slice(512 * t, 512 * (t + 1))
                        nc.vector.tensor_mul(
                            avn[0 : DH + 1, sl], avu[:, half], pcbs[t][0:33, :]
                        )
                else:
                    # one bounce round-trip for both chunks
                    nc.sync.dma_start(out=recd[0:1, 0:1024], in_=rec[0:1, 0:1024])
                    dsl = recd[0:1, 0:1024]
                    nc.sync.dma_start(
                        out=bc[:, 0:1024],
                        in_=bass.AP(
                            tensor=dsl.tensor,
                            offset=dsl.offset,
                            ap=[[0, DH + 1]] + list(dsl.ap[1:]),
                        ),
                    )
                    # normalize muls both on Pool (SBUF-only operands; with
                    # the C-class off there are no accum issues to block, and
                    # this removes 0.7us from DVE's group-boundary burst that
                    # was starving the 3-buffer sim pool). PO_DELAY covers
                    # Pool's serial ~3.3us before the po reads avn.
                    for t in range(2):
                        ic = 2 * ip + t
                        sl = slice(hoff + IC * ic, hoff + IC * (ic + 1))
                        half = slice(512 * t, 512 * (t + 1))
                        nc.gpsimd.tensor_mul(
                            avn[0 : DH + 1, sl], avu[:, half], bc[:, half]
                        )
                if h == 1:
                    extra = PO_EXTRA_G6 if gidx == 6 else 0
                    po_q.append((slot + PO_DELAY + extra, gidx))

            def stage_epilogue(gidx):
                # Spread the epilogue over one piece per slot, parity-matched
                # so each piece lands on the slot where its engine has no exp
                # (ACT exps run on even slots, DVE exps on odd): the 0.7-1.1us
                # pieces then fit the exp engines' per-slot headroom instead
                # of forming a ~2.5us burst that starves the 3-buffer sim
                # pool and stalls the PE at every group boundary.
                h, ip = groups[gidx]
                st = gstate[gidx]
                hoff = N * h

                def p0(s):  # ACT: evacuate a-half + corr bias
                    nc.scalar.activation(
                        avu[:, 0:512],
                        st["ava"][0 : DH + 1, :],
                        IDENT,
                        bias=corrS[0 : DH + 1, h : h + 1],
                    )

                def p1(s):  # DVE: evacuate b-half + corr bias
                    nc.vector.tensor_scalar_add(
                        avu[:, 512:1024],
                        st["avb"][0 : DH + 1, :],
                        corrS[0 : DH + 1, h : h + 1],
                    )

                def p2(s):  # DVE: reciprocal of both halves' denominators
                    nc.vector.reciprocal_approx_fast(
                        rec[0:1, 0:1024], avu[0:1, 0:1024]
                    )

                def p3(s):  # sync: DRAM bounce broadcast
                    nc.sync.dma_start(out=recd[0:1, 0:1024], in_=rec[0:1, 0:1024])
                    dsl = recd[0:1, 0:1024]
                    nc.sync.dma_start(
                        out=bc[:, 0:1024],
                        in_=bass.AP(
                            tensor=dsl.tensor,
                            offset=dsl.offset,
                            ap=[[0, DH + 1]] + list(dsl.ap[1:]),
                        ),
                    )

                def p4(s):  # Pool: normalize muls; then schedule po
                    for t in range(2):
                        ic = 2 * ip + t
                        sl = slice(hoff + IC * ic, hoff + IC * (ic + 1))
                        half = slice(512 * t, 512 * (t + 1))
                        nc.gpsimd.tensor_mul(
                            avn[0 : DH + 1, sl], avu[:, half], bc[:, half]
                        )
                    if h == 1:
                        extra = PO_EXTRA_G6 if gidx == 6 else 0
                        po_q.append((s + PO_DELAY + extra, gidx))

                ep_q.extend(
                    [("act", p0), ("dve", p1), ("dve", p2), ("any", p3), ("any", p4)]
                )

            def pump_ep_q(slot):
                # run at most one staged phase per slot, on the right parity
                if not ep_q:
                    return
                eng, fn = ep_q[0]
                if (
                    eng == "any"
                    or (eng == "act" and slot % 2 == 1)
                    or (eng == "dve" and slot % 2 == 0)
                ):
                    ep_q.pop(0)
                    fn(slot)

            def emit_po(gidx, last=False):
                h, ip = groups[gidx]
                po = simpool.tile([128, 1024], F32, tag="sim", name=f"po{gidx}")
                for t in range(2):
                    ic = 2 * ip + t
                    nc.tensor.matmul(
                        po[:, 512 * t : 512 * t + IC],
                        lhsT=woT_s[:, 0:128],
                        rhs=avn[:, IC * ic : IC * (ic + 1)],
                        start=True,
                        stop=False,
                    )
                    nc.tensor.matmul(
                        po[:, 512 * t : 512 * t + IC],
                        lhsT=woT_s[:, 128:256],
                        rhs=avn[:, N + IC * ic : N + IC * (ic + 1)],
                        start=False,
                        stop=True,
                    )
                # bias is folded into the projection (avn row 0 = denom*recip
                # = 1, woT row 0 of block 0 is bo); evacuate the two chunks on
                # different engines; the last po also fans its DMAs across the
                # sync+scalar queues so the drain doesn't serialize.
                for t in range(2):
                    ic = 2 * ip + t
                    half = slice(512 * t, 512 * (t + 1))
                    eng = nc.scalar.copy if t == 0 else nc.vector.tensor_copy
                    eng(outs[:, half], po[:, half])
                    dq = nc.scalar if (last and t == 1) else nc.sync
                    dq.dma_start(
                        out=out_d[:, IC * ic : IC * (ic + 1)],
                        in_=outs[:, half],
                    )

            slot = 0
            for gidx, (h, ip) in enumerate(groups):
                ica, icb = 2 * ip, 2 * ip + 1
                for js in range(NJS):
                    while po_q and po_q[0][0] <= slot:
                        emit_po(po_q.pop(0)[1])
                    pump_ep_q(slot)
                    if gidx == 0:
                        nc.gpsimd.memset(
                            _ap3(vts[js][:, 0:1], 33, 2, 1, 1), 1.0
                        )
                        if js < NPAIR:
                            for hh in range(2):
                                nc.gpsimd.memset(
                                    _ap3(vtp[js][hh][:, 0:1], 64, 2, 1, 1), 1.0
                                )
                    if USE_PACKED_PROJ:
                        if h == 0 and ip == 0 and js < 12 and js % 2 == 0:
                            qk_proj_one(2 + js // 2)
                    else:
                        if h == 0 and ip == 0 and js < NICH - 2:
                            qk_proj_one(0, js + 2)
                        if h == 0 and ip in (1, 2) and js % 8 == 0:
                            qk_proj_one(1, 4 * (ip - 1) + js // 8)
                    if h == 0 and ip == 0 and js < NJS - VLEAD:
                        v_proj_one(js + VLEAD)
                    if h == 0 and ip == 1 and js == 0:
                        emit_corr()
                    sim = simpool.tile([128, 1024], F32, tag="sim")
                    for t, icx in enumerate((ica, icb)):
                        if USE_PACKED_PROJ:
                            s_lhs = ktk[32 * h : 32 * h + 32, JS * js : JS * (js + 1)]
                            s_rhs = qkq[32 * h : 32 * h + 32, IC * icx : IC * (icx + 1)]
                        else:
                            s_lhs = kts[h][:, JS * js : JS * (js + 1)]
                            s_rhs = qts[h][:, IC * icx : IC * (icx + 1)]
                        nc.tensor.matmul(
                            sim[:, IC * t : IC * (t + 1)],
                            lhsT=s_lhs,
                            rhs=s_rhs,
                            start=True,
                            stop=True,
                        )
                    if js in _BSET:
                        pt = pts.tile([128, 1024], BF16, tag="pt")
                        nc.scalar.activation(pt[:], sim[:], EXP)
                        av_q.append(("b", pt, js, gidx))
                    else:
                        k = _DRSET.index(js)
                        pair, halfi = k // 2, k % 2
                        st = gstate[gidx]
                        if halfi == 0:
                            st["ptp"][pair] = ptps.tile(
                                [128, 2048], FP8, tag="ptp",
                                name=f"ptp{gidx}_{pair}",
                            )
                            # prefill any C-class halves of this pair with
                            # -1.0 so the casting accum DMA lands expm1
                            for hi in range(2):
                                if _DRSET[2 * pair + hi] in _CSET:
                                    nc.sync.dma_start(
                                        out=st["ptp"][pair][
                                            :, 1024 * hi : 1024 * (hi + 1)
                                        ],
                                        in_=neg1_d[:],
                                    )
                        ptile = st["ptp"][pair]
                        dst = ptile[:, 1024 * halfi : 1024 * (halfi + 1)]
                        if js in _CSET:
                            pt = pts.tile([128, 1024], BF16, tag="pt")
                            nc.scalar.activation(pt[:], sim[:], EXP)
                            nc.gpsimd.dma_start(out=dst, in_=pt[:], accum_op=ADD)
                        else:
                            nc.vector._custom_dve(
                                _EXPM1_OP, out=dst, in0=sim[:],
                                s0=_EXPC2, s1=_EXPC3, imm2=_EXPC4,
                            )
                        if halfi == 1:
                            av_q.append(("dr", ptile, pair, gidx))
                    if len(av_q) > AVLAG:
                        entry = av_q.pop(0)
                        if emit_av(entry):
                            g = entry[-1]
                            if g in PE_BCAST_GROUPS:
                                emit_epilogue(g, slot)
                            else:
                                stage_epilogue(g)
                    slot += 1

            while av_q:
                pump_ep_q(slot)
                entry = av_q.pop(0)
                if emit_av(entry):
                    g = entry[-1]
                    if g in PE_BCAST_GROUPS:
                        emit_epilogue(g, slot)
                    else:
                        stage_epilogue(g)
                slot += 1
            while ep_q:
                ep_q.pop(0)[1](slot)
                slot += 1
            while po_q:
                emit_po(po_q.pop(0)[1], last=(len(po_q) == 0))
    nc.finalize()
    return nc


_nc_cache = None


def _get_nc():
    global _nc_cache
    if _nc_cache is None:
        _nc_cache = _build()
    return _nc_cache


def make_in_maps(x, wq, wk, wv, wo, bo):
    b = 4
    xt = np.asarray(x, np.float32).reshape(b, C, N)
    wq = np.asarray(wq, np.float32)
    wk = np.asarray(wk, np.float32)
    wv = np.asarray(wv, np.float32)
    wo = np.asarray(wo, np.float32)
    bo = np.asarray(bo, np.float32)
    scale = DH ** (-0.5)

    def bf(a):
        return np.ascontiguousarray(a.astype(ml_dtypes.bfloat16))

    neg1 = np.full((C, 1024), -1.0, dtype=ml_dtypes.float8_e4m3fn)
    in_maps = []
    for core in range(8):
        bi, hp = core // 2, core % 2
        wq2 = wq[64 * hp : 64 * hp + 64] * scale
        wk2 = wk[64 * hp : 64 * hp + 64]
        wv2 = wv[64 * hp : 64 * hp + 64]
        if USE_PACKED_PROJ:
            # packed stationary: [Qh0|Qh1|Kh0|Kh1], 32 cols each
            wqkT = np.zeros((C, 128), np.float32)
            wqkT[:, 0:32] = wq2.T[:, 0:32]
            wqkT[:, 32:64] = wq2.T[:, 32:64]
            wqkT[:, 64:96] = wk2.T[:, 0:32]
            wqkT[:, 96:128] = wk2.T[:, 32:64]
        else:
            # v3: four zero-padded 128-col blocks [Qh0|Qh1|Kh0|Kh1]
            wqkT = np.zeros((C, 512), np.float32)
            wqkT[:, 0:32] = wq2.T[:, 0:32]
            wqkT[:, 128:160] = wq2.T[:, 32:64]
            wqkT[:, 256:288] = wk2.T[:, 0:32]
            wqkT[:, 384:416] = wk2.T[:, 32:64]
        wvT = np.zeros((C, VBLK), np.float32)  # cols 0,33 stay 0 (ones in SBUF)
        wvT[:, 1:33] = wv2.T[:, 0:32]
        wvT[:, 34:66] = wv2.T[:, 32:64]
        woT = np.zeros((C, 256), np.float32)
        woT[1:33, 0:128] = wo[:, 64 * hp : 64 * hp + 32].T
        woT[1:33, 128:256] = wo[:, 64 * hp + 32 : 64 * hp + 64].T
        if hp == 0:
            woT[0, 0:128] = bo  # bias rides avn row 0 (= denom/denom = 1)
        in_maps.append(
            {
                "xt": bf(xt[bi]),
                "wqkT": bf(wqkT),
                "wvT": bf(wvT),
                "woT": bf(woT),
                "neg1": neg1,
            }
        )
    return in_maps


def kernel(x, wq, wk, wv, wo, bo):
    global _last_results
    in_maps = make_in_maps(x, wq, wk, wv, wo, bo)
    nc = _get_nc()
    res = run_bass_kernel_spmd(nc, in_maps, core_ids=list(range(8)))
    _last_results = res
    outs = res.results
    out = np.zeros((4, C, N), np.float32)
    for bi in range(4):
        out[bi] = np.asarray(outs[2 * bi]["out"], np.float32) + np.asarray(
            outs[2 * bi + 1]["out"], np.float32
        )
    return out.reshape(4, C, 64, 64)
